# revision 5
# baseline (speedup 1.0000x reference)
"""Distributed multi-head attention kernel for 8 TRN2 NeuronCores.

Problem: B=2, S=2048, H=1024 (16 heads x 64), fp32 in/out.
Sharding: core c = 4*b + g handles batch b and head-group g (4 heads, 256
hidden cols). Wq/Wk/Wv column-sharded, Wo row-sharded; a per-q-chunk
bf16 ReduceScatter over each 4-core batch group yields each core's
4x128-row slices of the output.

v3: fully software-pipelined emission. Host pre-transposes x; xT loads
stream in 512-column chunks over the sync+scalar DMA queues so the
first projection chain starts ~8us in. K/V/Q/output projections run as
"filler" chains interleaved into the attention inner loop so the PE
pipeline never drains (a saturated PE queue sustains ~2x the
per-instruction rate). Softmax normalization broadcasts 1/sums via
gpsimd partition_broadcast, keeping the tensor queue free at loop
transitions. The exp() stream on ACT (~136us) and the PE stream
(~140us) are co-critical.

Dataflow per core (transpose-free attention, bf16 matmuls, fp32 PSUM):
  Q^T,K^T = (W^T x^T) in [j,t] layout; V = x^T-stationary @ Wv
  scores^T[k,q] = K^T.T@Q^T, two heads packed into PE row halves (K=64)
  Pt = exp(scores/8) (scores ~ N(0,1): exact softmax, no max pass)
  ctx^T[d,q] (+ sums row via ones column in V) = [V|1].T @ Pt
  normalize: rinv = 1/sums (DVE), partition_broadcast (gpsimd), mul
  out partial[t,o] = ctx^T-stationary @ Wo -> bf16 -> chunked RS.
bq/bk applied on-device (DVE bias-add); bv/bo folded on host (exact:
out += bv@Wo + bo, since softmax rows sum to one).
"""

import sys

for p in ("/opt/trn_rl_repo",):
    if p not in sys.path:
        sys.path.insert(0, p)

from collections import deque
from contextlib import ExitStack

import ml_dtypes
import numpy as np

from concourse import bacc, mybir, tile
from concourse.bass import ds
from concourse.bass_utils import run_bass_kernel_spmd

F32 = mybir.dt.float32
BF16 = mybir.dt.bfloat16
AF = mybir.ActivationFunctionType

B, S, H = 2, 2048, 1024
NH, D = 16, 64
NCORES = 8
GROUPS = [[0, 1, 2, 3], [4, 5, 6, 7]]
JG = 256           # hidden cols per core (4 heads)
SO = S // 4        # 512 output rows per core after reduce-scatter

_cache = {}


def _build():
    nc = bacc.Bacc("TRN2", target_bir_lowering=False, debug=False,
                   num_devices=NCORES)
    xT_d = nc.dram_tensor("xT", [H, S], BF16, kind="ExternalInput")
    wq_d = nc.dram_tensor("wq", [H, JG], BF16, kind="ExternalInput")
    wk_d = nc.dram_tensor("wk", [H, JG], BF16, kind="ExternalInput")
    wv_d = nc.dram_tensor("wv", [H, JG], BF16, kind="ExternalInput")
    wo_d = nc.dram_tensor("wo", [JG, H], BF16, kind="ExternalInput")
    bq_d = nc.dram_tensor("bqc", [128, 2], F32, kind="ExternalInput")
    bk_d = nc.dram_tensor("bkc", [128, 2], F32, kind="ExternalInput")
    out_d = nc.dram_tensor("out", [SO, H], BF16, kind="ExternalOutput")

    def mm(ps, lhsT, rhs, start, stop, tile_position=None):
        nc.tensor.matmul(ps, lhsT, rhs, start=start, stop=stop,
                         tile_position=tile_position)

    with tile.TileContext(nc) as tc, ExitStack() as st:
        consts = st.enter_context(tc.tile_pool(name="consts", bufs=1))
        ones1 = consts.tile([1, 64], BF16)
        nc.vector.memset(ones1[:], 1.0)
        bq_sb = consts.tile([128, 2], F32)
        bk_sb = consts.tile([128, 2], F32)

        wpool = st.enter_context(tc.tile_pool(name="weights", bufs=1))
        w_sb = {}

        def load_w(wname, wd, inner):
            # w*_sb[:, inner*s + j] = W[s*128 + p, j]
            wt = wpool.tile([128, 2048], BF16, name=f"{wname}sb",
                            tag=f"{wname}sb")
            nc.gpsimd.dma_start(
                wt[:].rearrange("p (s j) -> p s j", j=inner),
                wd.ap().rearrange("(s p) j -> p s j", p=128))
            w_sb[wname] = wt

        # deadline order: wq/wk feed the first projection chains
        load_w("wq", wq_d, 256)
        load_w("wk", wk_d, 256)
        load_w("wv", wv_d, 256)
        nc.gpsimd.dma_start(bq_sb[:], bq_d[:, :])
        nc.gpsimd.dma_start(bk_sb[:], bk_d[:, :])
        load_w("wo", wo_d, 1024)

        # x^T loads: 512-column chunks, chunk-major, split across the
        # sync and scalar hw DMA queues so chunk c lands at ~2.6*c us
        xTp = st.enter_context(tc.tile_pool(name="xT", bufs=1))
        xT = [xTp.tile([128, S], BF16, name=f"xT{s}", tag=f"xT{s}")
              for s in range(8)]
        for c in range(4):
            for s in range(8):
                eng = nc.sync if s < 4 else nc.scalar
                eng.dma_start(xT[s][:, ds(512 * c, 512)],
                              xT_d.ap()[ds(128 * s, 128), ds(512 * c, 512)])

        qkv = st.enter_context(tc.tile_pool(name="qkv", bufs=1))
        qT = [qkv.tile([128, S], BF16, name=f"qT{j}", tag=f"qT{j}")
              for j in range(2)]
        kT = [qkv.tile([128, S], BF16, name=f"kT{j}", tag=f"kT{j}")
              for j in range(2)]
        ctxT = [qkv.tile([128, S], BF16, name=f"cT{j}", tag=f"cT{j}")
                for j in range(2)]
        # V padded per head with a ones column: head h at cols 65h..65h+63
        v_sb = [qkv.tile([128, 260], BF16, name=f"v{i}", tag=f"v{i}")
                for i in range(16)]

        dram = st.enter_context(tc.tile_pool(name="dram", bufs=1, space="DRAM"))
        partial_c = [dram.tile([512, H], BF16, name=f"pc{i}", tag=f"pc{i}")
                     for i in range(4)]
        rs_c = [dram.tile([128, H], BF16, name=f"rc{i}", tag=f"rc{i}")
                for i in range(4)]

        with tc.tile_pool(name="scps", bufs=2, space="PSUM") as scps, \
             tc.tile_pool(name="ctxps", bufs=2, space="PSUM") as ctxps, \
             tc.tile_pool(name="fillps", bufs=2, space="PSUM") as fillps, \
             tc.tile_pool(name="psb", bufs=4) as psb, \
             tc.tile_pool(name="nrm", bufs=4) as nrm, \
             tc.tile_pool(name="osb", bufs=3) as osb:

            # ---- filler chains (run interleaved with attention) ----
            def qk_chain(which, jt, c):
                w = w_sb["wq"] if which == "q" else w_sb["wk"]
                dstT = qT if which == "q" else kT
                bias = bq_sb if which == "q" else bk_sb

                def emit():
                    ps = fillps.tile([128, 512], F32, tag="fill")
                    for s in range(8):
                        mm(ps[:], w[:, ds(256 * s + 128 * jt, 128)],
                           xT[s][:, ds(512 * c, 512)], s == 0, s == 7)
                    nc.vector.tensor_scalar_add(
                        dstT[jt][:, ds(512 * c, 512)], ps[:],
                        bias[:, ds(jt, 1)])
                return emit

            def v_chain(tv):
                def emit():
                    ps = fillps.tile([128, 512], F32, tag="fill")
                    for s in range(8):
                        mm(ps[:, 0:256], xT[s][:, ds(128 * tv, 128)],
                           w_sb["wv"][:, ds(256 * s, 256)], s == 0, s == 7)
                    nc.vector.memset(v_sb[tv][:], 1.0)
                    nc.vector.tensor_copy(
                        v_sb[tv][:].rearrange("p (h c) -> p h c", c=65)[:, :, 0:64],
                        ps[:, 0:256].rearrange("p (h c) -> p h c", c=64))
                return emit

            def o_chain(tq, tl):
                def emit():
                    tt = 4 * tq + tl
                    ot = osb.tile([128, 1024], BF16, tag="ot")
                    for oo in range(2):
                        ps = fillps.tile([128, 512], F32, tag="fill")
                        for idx, js in enumerate((1, 0)):
                            mm(ps[:], ctxT[js][:, ds(128 * tt, 128)],
                               w_sb["wo"][:, ds(1024 * js + 512 * oo, 512)],
                               idx == 0, idx == 1)
                        nc.vector.tensor_copy(ot[:, ds(512 * oo, 512)], ps[:])
                    nc.sync.dma_start(partial_c[tq][ds(128 * tl, 128), :],
                                      ot[:])
                return emit

            def rs_op(tq):
                def emit():
                    nc.gpsimd.collective_compute(
                        "ReduceScatter", mybir.AluOpType.add,
                        replica_groups=GROUPS,
                        ins=[partial_c[tq].opt()], outs=[rs_c[tq].opt()])
                return emit

            # pre-loop: Q(tq0), K(c0), V(tv0) emitted directly
            qk_chain("q", 0, 0)()
            qk_chain("q", 1, 0)()
            qk_chain("k", 0, 0)()
            qk_chain("k", 1, 0)()
            v_chain(0)()

            fillers = deque()
            fillers.append(v_chain(1))
            fillers.append(qk_chain("k", 0, 1))
            fillers.append(qk_chain("k", 1, 1))
            fillers.append(v_chain(2))
            fillers.append(v_chain(3))
            fillers.append(qk_chain("k", 0, 2))
            fillers.append(qk_chain("k", 1, 2))
            fillers.append(v_chain(4))
            fillers.append(v_chain(5))
            fillers.append(qk_chain("k", 0, 3))
            fillers.append(qk_chain("k", 1, 3))
            for tv in range(6, 16):
                fillers.append(v_chain(tv))
            fillers.append(qk_chain("q", 0, 1))
            fillers.append(qk_chain("q", 1, 1))

            for li, (tq, hp) in enumerate(
                    (t, h) for t in range(4) for h in (0, 1)):
                if tq >= 1 and hp == 0 and tq < 3:
                    fillers.append(qk_chain("q", 0, tq + 1))
                    fillers.append(qk_chain("q", 1, tq + 1))
                cA = ctxps.tile([65, 512], F32, tag="cps")
                cB = ctxps.tile([65, 512], F32, tag="cps")
                for kt in range(16):
                    sp = scps.tile([128, 1024], F32, tag="sp")
                    mm(sp[:, 0:512],
                       kT[hp][0:64, ds(128 * kt, 128)],
                       qT[hp][0:64, ds(512 * tq, 512)],
                       True, True, tile_position=(0, 0))
                    mm(sp[:, 512:1024],
                       kT[hp][64:128, ds(128 * kt, 128)],
                       qT[hp][64:128, ds(512 * tq, 512)],
                       True, True, tile_position=(64, 0))
                    pt = psb.tile([128, 1024], BF16, tag="pt")
                    nc.scalar.activation(pt[:], sp[:], AF.Exp, scale=0.125)
                    mm(cA[:], v_sb[kt][:, ds(65 * (2 * hp), 65)],
                       pt[:, 0:512], kt == 0, kt == 15)
                    mm(cB[:], v_sb[kt][:, ds(65 * (2 * hp + 1), 65)],
                       pt[:, 512:1024], kt == 0, kt == 15)
                    # fillers: none in the first kts (lets the loop
                    # transition settle so the tensor queue head never
                    # blocks on a filler's upstream DVE dependency)
                    budget = 2 if li == 0 else (0 if kt < 2 else 1)
                    for _ in range(budget):
                        if fillers:
                            fillers.popleft()()
                # normalize: broadcast raw sums via K=1 matmul (both heads'
                # sum-casts emitted first so the two bc matmuls run
                # back-to-back with minimal tensor-queue stall), then wide
                # reciprocal + multiply on DVE
                sms, bcs = [], []
                for cps in (cA, cB):
                    sm16 = nrm.tile([1, 512], BF16, tag="sm")
                    nc.vector.tensor_copy(sm16[:], cps[ds(64, 1), :])
                    sms.append(sm16)
                for sm16 in sms:
                    bc = fillps.tile([128, 512], F32, tag="fill")
                    mm(bc[0:64, :], ones1[:], sm16[:], True, True)
                    bcs.append(bc)
                for h, cps, bc in ((2 * hp, cA, bcs[0]), (2 * hp + 1, cB, bcs[1])):
                    rbc = nrm.tile([64, 512], F32, tag="rbc")
                    nc.vector.reciprocal_approx_fast(rbc[:], bc[0:64, :])
                    nc.vector.tensor_mul(
                        ctxT[hp][ds(64 * (h % 2), 64), ds(512 * tq, 512)],
                        cps[0:64, :], rbc[:])
                if hp == 1:
                    for tl in range(4):
                        fillers.append(o_chain(tq, tl))
                    fillers.append(rs_op(tq))

            while fillers:
                fillers.popleft()()

            # final output DMAs (deferred so they never block a queue)
            for tq in range(4):
                nc.sync.dma_start(out_d[ds(128 * tq, 128), :], rs_c[tq][:])

    nc.compile()
    return nc


def _get_nc():
    if "nc" not in _cache:
        _cache["nc"] = _build()
    return _cache["nc"]


def _in_maps(x, Wq, bq, Wk, bk, Wv, bv, Wo, bo):
    bf = ml_dtypes.bfloat16
    maps = []
    for c in range(NCORES):
        b, g = c // 4, c % 4
        j0 = JG * g
        maps.append({
            "xT": np.ascontiguousarray(x[b].T).astype(bf),
            "wq": np.ascontiguousarray(Wq[:, j0:j0 + JG]).astype(bf),
            "wk": np.ascontiguousarray(Wk[:, j0:j0 + JG]).astype(bf),
            "wv": np.ascontiguousarray(Wv[:, j0:j0 + JG]).astype(bf),
            "wo": np.ascontiguousarray(Wo[j0:j0 + JG, :]).astype(bf),
            "bqc": np.ascontiguousarray(bq[j0:j0 + JG].reshape(2, 128).T),
            "bkc": np.ascontiguousarray(bk[j0:j0 + JG].reshape(2, 128).T),
        })
    return maps


def kernel(x, Wq, bq, Wk, bk, Wv, bv, Wo, bo, _trace=False):
    x, Wq, bq, Wk, bk, Wv, bv, Wo, bo = (
        np.asarray(a, dtype=np.float32)
        for a in (x, Wq, bq, Wk, bk, Wv, bv, Wo, bo))
    nc = _get_nc()
    res = run_bass_kernel_spmd(nc, _in_maps(x, Wq, bq, Wk, bk, Wv, bv, Wo, bo),
                               core_ids=list(range(NCORES)), trace=_trace)
    out = np.empty((B, S, H), np.float32)
    for c in range(NCORES):
        b, g = c // 4, c % 4
        oc = np.asarray(res.results[c]["out"], dtype=np.float32)
        for tq in range(4):
            out[b, 512 * tq + 128 * g:512 * tq + 128 * (g + 1), :] = \
                oc[128 * tq:128 * (tq + 1)]
    out += bv @ Wo + bo  # exact: softmax rows sum to 1
    if _trace:
        return out, res
    return out


# revision 10
# speedup vs baseline: 1.0679x; 1.0679x over previous
"""Distributed multi-head attention kernel for 8 TRN2 NeuronCores.

Problem: B=2, S=2048, H=1024 (16 heads x 64), fp32 in/out.
Sharding: core c = 4*b + g handles batch b and head-group g (4 heads, 256
hidden cols). Wq/Wk/Wv column-sharded, Wo row-sharded; a per-q-chunk
bf16 ReduceScatter over each 4-core batch group yields each core's
4x128-row slices of the output.

v3: fully software-pipelined emission. Host pre-transposes x; xT loads
stream in 512-column chunks over the sync+scalar DMA queues so the
first projection chain starts ~8us in. K/V/Q/output projections run as
"filler" chains interleaved into the attention inner loop so the PE
pipeline never drains (a saturated PE queue sustains ~2x the
per-instruction rate). Softmax normalization broadcasts 1/sums via
gpsimd partition_broadcast, keeping the tensor queue free at loop
transitions. The exp() stream on ACT (~136us) and the PE stream
(~140us) are co-critical.

Dataflow per core (transpose-free attention, bf16 matmuls, fp32 PSUM):
  Q^T,K^T = (W^T x^T) in [j,t] layout; V = x^T-stationary @ Wv
  scores^T[k,q] = K^T.T@Q^T, two heads packed into PE row halves (K=64)
  Pt = exp(scores/8) (scores ~ N(0,1): exact softmax, no max pass)
  ctx^T[d,q] (+ sums row via ones column in V) = [V|1].T @ Pt
  normalize: rinv = 1/sums (DVE), partition_broadcast (gpsimd), mul
  out partial[t,o] = ctx^T-stationary @ Wo -> bf16 -> chunked RS.
bq/bk applied on-device (DVE bias-add); bv/bo folded on host (exact:
out += bv@Wo + bo, since softmax rows sum to one).
"""

import sys

for p in ("/opt/trn_rl_repo",):
    if p not in sys.path:
        sys.path.insert(0, p)

from collections import deque
from contextlib import ExitStack

import ml_dtypes
import numpy as np

from concourse import bacc, mybir, tile
from concourse.bass import ds
from concourse.bass_utils import run_bass_kernel_spmd

F32 = mybir.dt.float32
BF16 = mybir.dt.bfloat16
AF = mybir.ActivationFunctionType

B, S, H = 2, 2048, 1024
NH, D = 16, 64
NCORES = 8
GROUPS = [[0, 1, 2, 3], [4, 5, 6, 7]]
JG = 256           # hidden cols per core (4 heads)
SO = S // 4        # 512 output rows per core after reduce-scatter

_cache = {}


def _build():
    nc = bacc.Bacc("TRN2", target_bir_lowering=False, debug=False,
                   num_devices=NCORES)
    xT_d = nc.dram_tensor("xT", [H, S], BF16, kind="ExternalInput")
    wq_d = nc.dram_tensor("wq", [H, JG], BF16, kind="ExternalInput")
    wk_d = nc.dram_tensor("wk", [H, JG], BF16, kind="ExternalInput")
    wv_d = nc.dram_tensor("wv", [H, JG], BF16, kind="ExternalInput")
    wo_d = nc.dram_tensor("wo", [JG, H], BF16, kind="ExternalInput")
    bq_d = nc.dram_tensor("bqc", [128, 2], F32, kind="ExternalInput")
    bk_d = nc.dram_tensor("bkc", [128, 2], F32, kind="ExternalInput")
    out_d = nc.dram_tensor("out", [SO, H], BF16, kind="ExternalOutput")

    def mm(ps, lhsT, rhs, start, stop, tile_position=None):
        nc.tensor.matmul(ps, lhsT, rhs, start=start, stop=stop,
                         tile_position=tile_position)

    with tile.TileContext(nc) as tc, ExitStack() as st:
        consts = st.enter_context(tc.tile_pool(name="consts", bufs=1))
        ones1 = consts.tile([1, 64], BF16)
        nc.vector.memset(ones1[:], 1.0)
        bq_sb = consts.tile([128, 2], F32)
        bk_sb = consts.tile([128, 2], F32)

        wpool = st.enter_context(tc.tile_pool(name="weights", bufs=1))
        w_sb = {}

        def load_w(wname, wd, inner):
            # w*_sb[:, inner*s + j] = W[s*128 + p, j]
            wt = wpool.tile([128, 2048], BF16, name=f"{wname}sb",
                            tag=f"{wname}sb")
            nc.gpsimd.dma_start(
                wt[:].rearrange("p (s j) -> p s j", j=inner),
                wd.ap().rearrange("(s p) j -> p s j", p=128))
            w_sb[wname] = wt

        # deadline order: wq/wk feed the first projection chains
        load_w("wq", wq_d, 256)
        load_w("wk", wk_d, 256)
        load_w("wv", wv_d, 256)
        nc.gpsimd.dma_start(bq_sb[:], bq_d[:, :])
        nc.gpsimd.dma_start(bk_sb[:], bk_d[:, :])
        load_w("wo", wo_d, 1024)

        # x^T loads: 512-column chunks, chunk-major, split across the
        # sync and scalar hw DMA queues so chunk c lands at ~2.6*c us
        xTp = st.enter_context(tc.tile_pool(name="xT", bufs=1))
        xT = [xTp.tile([128, S], BF16, name=f"xT{s}", tag=f"xT{s}")
              for s in range(8)]
        for c in range(4):
            for s in range(8):
                eng = nc.sync if s < 4 else nc.scalar
                eng.dma_start(xT[s][:, ds(512 * c, 512)],
                              xT_d.ap()[ds(128 * s, 128), ds(512 * c, 512)])

        qkv = st.enter_context(tc.tile_pool(name="qkv", bufs=1))
        qT = [qkv.tile([128, S], BF16, name=f"qT{j}", tag=f"qT{j}")
              for j in range(2)]
        kT = [qkv.tile([128, S], BF16, name=f"kT{j}", tag=f"kT{j}")
              for j in range(2)]
        ctxT = [qkv.tile([128, S], BF16, name=f"cT{j}", tag=f"cT{j}")
                for j in range(2)]
        # V padded per head with a ones column: head h at cols 65h..65h+63
        v_sb = [qkv.tile([128, 260], BF16, name=f"v{i}", tag=f"v{i}")
                for i in range(16)]

        dram = st.enter_context(tc.tile_pool(name="dram", bufs=1, space="DRAM"))
        partial_c = [dram.tile([512, H], BF16, name=f"pc{i}", tag=f"pc{i}")
                     for i in range(4)]
        rs_c = [dram.tile([128, H], BF16, name=f"rc{i}", tag=f"rc{i}")
                for i in range(4)]

        with tc.tile_pool(name="scps", bufs=2, space="PSUM") as scps, \
             tc.tile_pool(name="ctxps", bufs=2, space="PSUM") as ctxps, \
             tc.tile_pool(name="fillps", bufs=2, space="PSUM") as fillps, \
             tc.tile_pool(name="psb", bufs=4) as psb, \
             tc.tile_pool(name="nrm", bufs=4) as nrm, \
             tc.tile_pool(name="osb", bufs=3) as osb:

            # ---- filler chains (run interleaved with attention) ----
            def qk_chain(which, jt, c):
                w = w_sb["wq"] if which == "q" else w_sb["wk"]
                dstT = qT if which == "q" else kT
                bias = bq_sb if which == "q" else bk_sb

                def emit():
                    ps = fillps.tile([128, 512], F32, tag="fill")
                    for s in range(8):
                        mm(ps[:], w[:, ds(256 * s + 128 * jt, 128)],
                           xT[s][:, ds(512 * c, 512)], s == 0, s == 7)
                    nc.vector.tensor_scalar_add(
                        dstT[jt][:, ds(512 * c, 512)], ps[:],
                        bias[:, ds(jt, 1)])
                return emit

            def v_chain(tv):
                def emit():
                    ps = fillps.tile([128, 512], F32, tag="fill")
                    for s in range(8):
                        mm(ps[:, 0:256], xT[s][:, ds(128 * tv, 128)],
                           w_sb["wv"][:, ds(256 * s, 256)], s == 0, s == 7)
                    nc.vector.memset(v_sb[tv][:], 1.0)
                    nc.vector.tensor_copy(
                        v_sb[tv][:].rearrange("p (h c) -> p h c", c=65)[:, :, 0:64],
                        ps[:, 0:256].rearrange("p (h c) -> p h c", c=64))
                return emit

            stages = {}

            def o_chain(tq, tl):
                def emit():
                    tt = 4 * tq + tl
                    if tl == 0:
                        stages[tq] = osb.tile([128, 4096], BF16,
                                              name=f"ot{tq}", tag="ot")
                    stage = stages[tq]
                    for oo in range(2):
                        ps = fillps.tile([128, 512], F32, tag="fill")
                        for idx, js in enumerate((1, 0)):
                            mm(ps[:], ctxT[js][:, ds(128 * tt, 128)],
                               w_sb["wo"][:, ds(1024 * js + 512 * oo, 512)],
                               idx == 0, idx == 1)
                        nc.vector.tensor_copy(
                            stage[:, ds(1024 * tl + 512 * oo, 512)], ps[:])
                    if tl == 3:
                        # one DMA for the whole 512-row chunk: the RS
                        # trigger then waits on a single semaphore
                        nc.sync.dma_start(
                            partial_c[tq][:].rearrange(
                                "(tl p) o -> p tl o", p=128),
                            stage[:].rearrange("p (tl o) -> p tl o", o=1024))
                return emit

            def rs_op(tq):
                def emit():
                    nc.gpsimd.collective_compute(
                        "ReduceScatter", mybir.AluOpType.add,
                        replica_groups=GROUPS,
                        ins=[partial_c[tq].opt()], outs=[rs_c[tq].opt()])
                return emit

            # warm up the PE while the xT/weight DMAs stream: a cold
            # tensor engine runs at less than half rate for its first
            # ~3us, so burn that ramp on junk matmuls during the loads
            junk = consts.tile([128, 512], BF16, name="junk", tag="junk")
            nc.vector.memset(junk[:], 0.0)
            for i in range(14):
                wp = scps.tile([128, 1024], F32, tag="sp")
                mm(wp[:, 0:512], junk[:, 0:128], junk[:], True, True)

            # pre-loop: Q(tq0), K(c0), V(tv0) emitted directly
            qk_chain("q", 0, 0)()
            qk_chain("q", 1, 0)()
            qk_chain("k", 0, 0)()
            qk_chain("k", 1, 0)()
            v_chain(0)()

            fillers = deque()
            fillers.append(v_chain(1))
            fillers.append(qk_chain("k", 0, 1))
            fillers.append(qk_chain("k", 1, 1))
            fillers.append(v_chain(2))
            fillers.append(v_chain(3))
            fillers.append(qk_chain("k", 0, 2))
            fillers.append(qk_chain("k", 1, 2))
            fillers.append(v_chain(4))
            fillers.append(v_chain(5))
            fillers.append(qk_chain("k", 0, 3))
            fillers.append(qk_chain("k", 1, 3))
            for tv in range(6, 16):
                fillers.append(v_chain(tv))
            fillers.append(qk_chain("q", 0, 1))
            fillers.append(qk_chain("q", 1, 1))

            def emit_norm(pending):
                # previous loop's normalize: broadcast raw sums via K=1
                # matmuls, wide reciprocal + multiply on DVE
                ptq, php, pcA, pcB, sms = pending
                bcs = []
                for sm16 in sms:
                    bc = fillps.tile([128, 512], F32, tag="fill")
                    mm(bc[0:64, :], ones1[:], sm16[:], True, True)
                    bcs.append(bc)
                for h, cps, bc in ((2 * php, pcA, bcs[0]),
                                   (2 * php + 1, pcB, bcs[1])):
                    rbc = nrm.tile([64, 512], F32, tag="rbc")
                    nc.vector.reciprocal_approx_fast(rbc[:], bc[0:64, :])
                    nc.vector.tensor_mul(
                        ctxT[php][ds(64 * (h % 2), 64), ds(512 * ptq, 512)],
                        cps[0:64, :], rbc[:])

            pending = None
            for li, (tq, hp) in enumerate(
                    (t, h) for t in range(4) for h in (0, 1)):
                if tq >= 1 and hp == 0 and tq < 3:
                    fillers.append(qk_chain("q", 0, tq + 1))
                    fillers.append(qk_chain("q", 1, tq + 1))
                cA = ctxps.tile([65, 512], F32, tag="cps")
                cB = ctxps.tile([65, 512], F32, tag="cps")
                deferred_ctx = []
                for kt in range(16):
                    sp = scps.tile([128, 1024], F32, tag="sp")
                    mm(sp[:, 0:512],
                       kT[hp][0:64, ds(128 * kt, 128)],
                       qT[hp][0:64, ds(512 * tq, 512)],
                       True, True, tile_position=(0, 0))
                    mm(sp[:, 512:1024],
                       kT[hp][64:128, ds(128 * kt, 128)],
                       qT[hp][64:128, ds(512 * tq, 512)],
                       True, True, tile_position=(64, 0))
                    pt = psb.tile([128, 1024], BF16, tag="pt")
                    nc.scalar.activation(pt[:], sp[:], AF.Exp, scale=0.125)

                    def ctx_pair(kt, pt):
                        mm(cA[:], v_sb[kt][:, ds(65 * (2 * hp), 65)],
                           pt[:, 0:512], kt == 0, kt == 15)
                        mm(cB[:], v_sb[kt][:, ds(65 * (2 * hp + 1), 65)],
                           pt[:, 512:1024], kt == 0, kt == 15)

                    # kt 0/1: emit scores+exp only, then the previous
                    # loop's normalize, then the deferred ctx pairs — the
                    # ACT stream never waits on the loop transition
                    if li > 0 and kt < 2:
                        deferred_ctx.append((kt, pt))
                        if kt == 1:
                            if pending is not None:
                                emit_norm(pending)
                                pending = None
                            for a in deferred_ctx:
                                ctx_pair(*a)
                            deferred_ctx = []
                    else:
                        ctx_pair(kt, pt)
                    budget = 2 if li == 0 else (0 if kt < 2 else 1)
                    for _ in range(budget):
                        if fillers:
                            fillers.popleft()()
                # cast both heads' sums rows now (DVE); the bc matmuls and
                # multiplies run early in the next loop
                sms = []
                for cps in (cA, cB):
                    sm16 = nrm.tile([1, 512], BF16, tag="sm")
                    nc.vector.tensor_copy(sm16[:], cps[ds(64, 1), :])
                    sms.append(sm16)
                pending = (tq, hp, cA, cB, sms)
                if hp == 1:
                    for tl in range(4):
                        fillers.append(o_chain(tq, tl))
                    fillers.append(rs_op(tq))

            emit_norm(pending)
            while fillers:
                fillers.popleft()()

            # final output DMAs (deferred so they never block a queue)
            for tq in range(4):
                nc.sync.dma_start(out_d[ds(128 * tq, 128), :], rs_c[tq][:])

    nc.compile()
    return nc


def _get_nc():
    if "nc" not in _cache:
        _cache["nc"] = _build()
    return _cache["nc"]


def _in_maps(x, Wq, bq, Wk, bk, Wv, bv, Wo, bo):
    bf = ml_dtypes.bfloat16
    maps = []
    for c in range(NCORES):
        b, g = c // 4, c % 4
        j0 = JG * g
        maps.append({
            "xT": np.ascontiguousarray(x[b].T).astype(bf),
            "wq": np.ascontiguousarray(Wq[:, j0:j0 + JG]).astype(bf),
            "wk": np.ascontiguousarray(Wk[:, j0:j0 + JG]).astype(bf),
            "wv": np.ascontiguousarray(Wv[:, j0:j0 + JG]).astype(bf),
            "wo": np.ascontiguousarray(Wo[j0:j0 + JG, :]).astype(bf),
            "bqc": np.ascontiguousarray(bq[j0:j0 + JG].reshape(2, 128).T),
            "bkc": np.ascontiguousarray(bk[j0:j0 + JG].reshape(2, 128).T),
        })
    return maps


def kernel(x, Wq, bq, Wk, bk, Wv, bv, Wo, bo, _trace=False):
    x, Wq, bq, Wk, bk, Wv, bv, Wo, bo = (
        np.asarray(a, dtype=np.float32)
        for a in (x, Wq, bq, Wk, bk, Wv, bv, Wo, bo))
    nc = _get_nc()
    res = run_bass_kernel_spmd(nc, _in_maps(x, Wq, bq, Wk, bk, Wv, bv, Wo, bo),
                               core_ids=list(range(NCORES)), trace=_trace)
    out = np.empty((B, S, H), np.float32)
    for c in range(NCORES):
        b, g = c // 4, c % 4
        oc = np.asarray(res.results[c]["out"], dtype=np.float32)
        for tq in range(4):
            out[b, 512 * tq + 128 * g:512 * tq + 128 * (g + 1), :] = \
                oc[128 * tq:128 * (tq + 1)]
    out += bv @ Wo + bo  # exact: softmax rows sum to 1
    if _trace:
        return out, res
    return out
